# revision 1
# baseline (speedup 1.0000x reference)
"""MoE routing kernel for TRN2 (8 NeuronCores).

The reference MoE applies row 0's top-2 expert choice (indices and softmax
weights) to the entire batch, so the whole module collapses to

    out = x @ (w0*We[i0] + w1*We[i1]).T + (w0*be[i0] + w1*be[i1])

a single [16384,2048] @ [2048,2048] matmul with bias. Host does the tiny
row-0 gating and combines the two selected experts; the device runs the
matmul data-parallel over tokens (2048 tokens per core, no collectives).

Per-core schedule (profile-driven):
  Stage 1: the first 4 m-tiles run k-outer in two n-pair phases using all
           8 PSUM banks, chasing the W DMA stream (W arrives as 16
           independent [128,4,512] chunks, j-major within each n-pair),
           so the PE starts ~12us in and tracks the stream.
  Stage 2: remaining 12 m-tiles run k-inner against the resident W; x
           arrives in a packed layout with 4KB DRAM runs.
Inputs stream on the SP HWDGE queue, outputs on the Activation queue.
float32r matmuls run at 1 cycle/row (full PE rate) with ~1e-4 rel err.
"""

import os
import sys

import numpy as np

if "/opt/trn_rl_repo" not in sys.path:
    sys.path.insert(0, "/opt/trn_rl_repo")

N, D, E, TOPK = 16384, 2048, 8, 2
N_CORES = 8
M_SHARD = N // N_CORES  # 2048 tokens per core
P = 128
K_TILES = D // P        # 16 contraction slabs
M_TILES = M_SHARD // P  # 16
N_FREE = 512
N_TILES = D // N_FREE   # 4
KG = 4                  # k-slabs per chunk
JG = K_TILES // KG      # 4 chunks per n-tile
M_HEAD = 4              # m-tiles computed during the W stream (stage 1)
M_SLAB = 256            # tokens per stage-2 slab (two m-tiles)
N_SLABS = (M_SHARD - M_HEAD * P) // M_SLAB  # 6

_CACHE = {}


def _build_nc():
    import concourse.tile as tile
    from concourse import bacc, mybir

    nc = bacc.Bacc(None, target_bir_lowering=False)
    f32 = mybir.dt.float32
    f32r = mybir.dt.float32r

    # DRAM I/O (packed layouts: 8KB/4KB contiguous runs per partition).
    xp = nc.dram_tensor("xp", [JG, P, KG, M_HEAD * P], f32r, kind="ExternalInput")
    xq = nc.dram_tensor("xq", [N_SLABS, JG, P, KG, M_SLAB], f32r,
                        kind="ExternalInput")
    wt = nc.dram_tensor("wt", [N_TILES, JG, P, KG, N_FREE], f32r,
                        kind="ExternalInput")
    bias = nc.dram_tensor("bias", [P, D], f32, kind="ExternalInput")
    out = nc.dram_tensor("out", [M_SHARD, D], f32, kind="ExternalOutput")

    with tile.TileContext(nc) as tc:
        with tc.tile_pool(name="wpool", bufs=1) as wpool, \
             tc.tile_pool(name="xppool", bufs=1) as xppool, \
             tc.tile_pool(name="xqpool", bufs=2) as xqpool, \
             tc.tile_pool(name="bpool", bufs=1) as bpool, \
             tc.tile_pool(name="opool", bufs=3) as opool, \
             tc.tile_pool(name="psum", bufs=1, space="PSUM") as psum_pool:

            wc = [[None] * JG for _ in range(N_TILES)]
            xpt = [None] * JG

            def load_w(n, j):
                w = wpool.tile([P, KG, N_FREE], f32r, name=f"w{n}{j}",
                               tag=f"w{n}_{j}")
                nc.sync.dma_start(out=w[:, :, :], in_=wt[n, j])
                wc[n][j] = w

            # n-pair (0,1) chunks j-major, interleaved with xp.
            for j in range(JG):
                load_w(0, j)
                t = xppool.tile([P, KG, M_HEAD * P], f32r, name=f"xp{j}",
                                tag=f"xp{j}")
                nc.sync.dma_start(out=t[:, :, :], in_=xp[j])
                xpt[j] = t
                load_w(1, j)

            bias_t = bpool.tile([P, D], f32, name="bias_t", tag="bias_t")
            nc.sync.dma_start(out=bias_t[:, :], in_=bias[:, :])

            xqt = [[None] * JG for _ in range(N_SLABS)]

            def load_slab(s):
                for j in range(JG):
                    t = xqpool.tile([P, KG, M_SLAB], f32r, name=f"xq{j}",
                                    tag=f"xq{j}")
                    nc.sync.dma_start(out=t[:, :, :], in_=xq[s, j])
                    xqt[s][j] = t

            load_slab(0)
            for j in range(JG):
                load_w(2, j)
                load_w(3, j)
            load_slab(1)

            def evict(ps, m, n):
                ot = opool.tile([P, N_FREE], f32, name="ot", tag="ot")
                nc.vector.tensor_add(
                    ot[:, :], ps[:, :],
                    bias_t[:, n * N_FREE:(n + 1) * N_FREE],
                )
                nc.scalar.dma_start(
                    out=out[m * P:(m + 1) * P, n * N_FREE:(n + 1) * N_FREE],
                    in_=ot[:, :],
                )

            # Stage 1: m0..3, two n-pair phases, j-outer k-chase.
            for phase in range(2):
                pss = {}
                for n in (2 * phase, 2 * phase + 1):
                    for m in range(M_HEAD):
                        pss[(n, m)] = psum_pool.tile(
                            [P, N_FREE], f32, name=f"ps{n}_{m}",
                            tag=f"ps{(n % 2) * 4 + m}")
                for j in range(JG):
                    for kk in range(KG):
                        for n in (2 * phase, 2 * phase + 1):
                            for m in range(M_HEAD):
                                nc.tensor.matmul(
                                    pss[(n, m)][:, :],
                                    lhsT=xpt[j][:, kk, m * P:(m + 1) * P],
                                    rhs=wc[n][j][:, kk, :],
                                    start=(j == 0 and kk == 0),
                                    stop=(j == JG - 1 and kk == KG - 1),
                                )
                for n in (2 * phase, 2 * phase + 1):
                    for m in range(M_HEAD):
                        evict(pss[(n, m)], m, n)

            # Stage 2: m4..15, k-inner against resident W.
            cnt = 0
            for s in range(N_SLABS):
                if s >= 2:
                    load_slab(s)
                for mi in range(M_SLAB // P):
                    m = M_HEAD + s * (M_SLAB // P) + mi
                    for n in range(N_TILES):
                        ps = psum_pool.tile([P, N_FREE], f32, name="ps2",
                                            tag=f"ps{cnt % 8}")
                        cnt += 1
                        for k in range(K_TILES):
                            nc.tensor.matmul(
                                ps[:, :],
                                lhsT=xqt[s][k // KG][:, k % KG,
                                                     mi * P:(mi + 1) * P],
                                rhs=wc[n][k // KG][:, k % KG, :],
                                start=(k == 0),
                                stop=(k == K_TILES - 1),
                            )
                        evict(ps, m, n)

    nc.compile()
    return nc


def _get_nc():
    if "nc" not in _CACHE:
        _CACHE["nc"] = _build_nc()
    return _CACHE["nc"]


def _ensure_ntff_hook():
    """Register the axon NTFF profile hook (the image's antenv lacks
    axon_hooks; recreate it and wire the ctypes hook from trn_boot)."""
    import types

    try:
        from antenv.axon_hooks import get_axon_ntff_profile_hook  # noqa: F401
        return
    except ImportError:
        pass
    try:
        import antenv
        from trn_agent_boot.trn_boot import _ntff_profile_via_ctypes

        mod = types.ModuleType("antenv.axon_hooks")
        _state = {"hook": None}
        mod.set_axon_ntff_profile_hook = lambda h: _state.__setitem__("hook", h)
        mod.get_axon_ntff_profile_hook = lambda: _state["hook"]
        sys.modules["antenv.axon_hooks"] = mod
        antenv.axon_hooks = mod
        mod.set_axon_ntff_profile_hook(
            _ntff_profile_via_ctypes("/opt/axon/libaxon_pjrt.so")
        )
        # avoid the S3 artifact upload in the trace path
        import concourse.bass_utils as bu

        bu.upload_artifacts = lambda tmpdir: tmpdir
    except Exception as e:  # profiling is best-effort
        print(f"NTFF hook setup failed: {e}", file=sys.stderr)


def kernel(x, Wg, bg, We, be):
    from concourse.bass_utils import run_bass_kernel_spmd

    x = np.asarray(x, dtype=np.float32)
    Wg = np.asarray(Wg, dtype=np.float32)
    bg = np.asarray(bg, dtype=np.float32)
    We = np.asarray(We, dtype=np.float32)
    be = np.asarray(be, dtype=np.float32)

    # Row-0 gating on host (16K FLOPs): softmax over 8 logits, top-2.
    logits = x[0].astype(np.float64) @ Wg.astype(np.float64).T + bg.astype(
        np.float64
    )
    probs = np.exp(logits - logits.max())
    probs /= probs.sum()
    idx = np.argsort(-probs, kind="stable")[:TOPK]
    w0 = probs[idx]

    Wc = w0[0] * We[idx[0]].astype(np.float64) + w0[1] * We[idx[1]].astype(
        np.float64
    )
    bc = w0[0] * be[idx[0]].astype(np.float64) + w0[1] * be[idx[1]].astype(
        np.float64
    )
    WcT = np.ascontiguousarray(Wc.T).astype(np.float32)  # [d, o]
    # [n, j, p, kk, f]: d = (j kk p), o = (n f)
    wt = np.ascontiguousarray(
        WcT.reshape(JG, KG, P, N_TILES, N_FREE).transpose(3, 0, 2, 1, 4)
    )
    bias = np.ascontiguousarray(
        np.broadcast_to(bc.astype(np.float32), (P, D))
    )

    nc = _get_nc()
    in_maps = []
    mh = M_HEAD * P
    for c in range(N_CORES):
        xsh = x[c * M_SHARD:(c + 1) * M_SHARD]           # [m, d]
        xT = np.ascontiguousarray(xsh.T)                 # [d, m]
        x5 = xT.reshape(JG, KG, P, M_SHARD)              # [j, kk, p, m]
        # head tokens packed [j, p, kk, m]
        xph = np.ascontiguousarray(x5[:, :, :, :mh].transpose(0, 2, 1, 3))
        # stage-2 slabs packed [s, j, p, kk, m]
        xqh = np.ascontiguousarray(
            x5[:, :, :, mh:].reshape(JG, KG, P, N_SLABS, M_SLAB)
            .transpose(3, 0, 2, 1, 4)
        )
        in_maps.append({"xp": xph, "xq": xqh, "wt": wt, "bias": bias})

    trace = bool(int(os.environ.get("KERNEL_TRACE", "0")))
    tmpdir = None
    if trace:
        import tempfile

        _ensure_ntff_hook()
        tmpdir = tempfile.mkdtemp(prefix="moe_trace_")
        _CACHE["last_tmpdir"] = tmpdir
    res = run_bass_kernel_spmd(
        nc, in_maps, core_ids=list(range(N_CORES)), trace=trace, tmpdir=tmpdir
    )
    _CACHE["last_results"] = res

    return np.concatenate(
        [res.results[c]["out"] for c in range(N_CORES)], axis=0
    )



# revision 2
# speedup vs baseline: 1.1339x; 1.1339x over previous
"""MoE routing kernel for TRN2 (8 NeuronCores).

The reference MoE applies row 0's top-2 expert choice (indices and softmax
weights) to the entire batch, so the whole module collapses to

    out = x @ (w0*We[i0] + w1*We[i1]).T + (w0*be[i0] + w1*be[i1])

a single [16384,2048] @ [2048,2048] matmul with bias. Host does the tiny
row-0 gating and combines the two selected experts; the device runs the
matmul data-parallel over tokens (2048 tokens per core, no collectives).

v2 schedule (trace-driven): inputs are bf16 (x stationary, W moving),
PSUM/bias/out stay fp32. A few warm-up matmuls on memset scratch ramp the
PE HAM throttle to K=8/8 before real data lands. Stage A chases the W
stream over m0..3 in two n-pair phases (8 PSUM banks each); stage B runs
m4..15 k-inner against the fully resident W. All inputs stream on the
sync (SP) queue in exactly consumption order; bias and outputs use the
scalar (Activation) queue.
"""

import os
import sys

import numpy as np

if "/opt/trn_rl_repo" not in sys.path:
    sys.path.insert(0, "/opt/trn_rl_repo")

N, D, E, TOPK = 16384, 2048, 8, 2
N_CORES = 8
M_SHARD = N // N_CORES  # 2048 tokens per core
P = 128
KT = D // P             # 16 contraction slabs
MT = M_SHARD // P       # 16 m tiles
NF = 512
NT = D // NF            # 4 n tiles
MA = 4                  # m-tiles covered in stage A
WARM_MMS = 6

_CACHE = {}


def _build_nc():
    import concourse.tile as tile
    from concourse import bacc, mybir

    nc = bacc.Bacc(None, target_bir_lowering=False)
    f32 = mybir.dt.float32
    bf16 = mybir.dt.bfloat16

    xA = nc.dram_tensor("xA", [KT, P, MA * P], bf16, kind="ExternalInput")
    xB = nc.dram_tensor("xB", [KT, P, (MT - MA) * P], bf16,
                        kind="ExternalInput")
    wt = nc.dram_tensor("wt", [2, KT, P, 2, NF], bf16, kind="ExternalInput")
    bias = nc.dram_tensor("bias", [P, D], f32, kind="ExternalInput")
    out = nc.dram_tensor("out", [M_SHARD, D], f32, kind="ExternalOutput")

    with tile.TileContext(nc) as tc:
        with tc.tile_pool(name="wpool", bufs=1) as wpool, \
             tc.tile_pool(name="xpool", bufs=1) as xpool, \
             tc.tile_pool(name="bpool", bufs=1) as bpool, \
             tc.tile_pool(name="warm", bufs=1) as warm_pool, \
             tc.tile_pool(name="opool", bufs=3) as opool, \
             tc.tile_pool(name="psum", bufs=1, space="PSUM") as psum_pool:

            # Warm-up: ramp the PE power state while the first DMAs fly.
            warm_w = warm_pool.tile([P, P], bf16, name="warm_w", tag="warm_w")
            warm_x = warm_pool.tile([P, NF], bf16, name="warm_x", tag="warm_x")
            nc.vector.memset(warm_w[:, :], 0.0)
            nc.vector.memset(warm_x[:, :], 0.0)
            ps_warm = psum_pool.tile([P, NF], f32, name="ps_warm", tag="b0")
            for _ in range(WARM_MMS):
                nc.tensor.matmul(ps_warm[:, :], lhsT=warm_w[:, :],
                                 rhs=warm_x[:, :], start=True, stop=True)

            # Input DMAs on the sync (SP) queue, in consumption order.
            wc = [[None] * KT for _ in range(2)]
            xAt = [None] * KT
            xBt = [None] * KT
            for k in range(KT):
                w = wpool.tile([P, 2, NF], bf16, name=f"w0_{k}",
                               tag=f"w0_{k}")
                nc.sync.dma_start(out=w[:, :, :], in_=wt[0, k])
                wc[0][k] = w
                t = xpool.tile([P, MA * P], bf16, name=f"xa{k}", tag=f"xa{k}")
                nc.sync.dma_start(out=t[:, :], in_=xA[k])
                xAt[k] = t
            for k in range(KT):
                w = wpool.tile([P, 2, NF], bf16, name=f"w1_{k}",
                               tag=f"w1_{k}")
                nc.sync.dma_start(out=w[:, :, :], in_=wt[1, k])
                wc[1][k] = w
            for k in range(KT):
                t = xpool.tile([P, (MT - MA) * P], bf16, name=f"xb{k}",
                               tag=f"xb{k}")
                nc.sync.dma_start(out=t[:, :], in_=xB[k])
                xBt[k] = t

            # Bias on the (otherwise idle early) scalar queue.
            bias_t = bpool.tile([P, D], f32, name="bias_t", tag="bias_t")
            nc.scalar.dma_start(out=bias_t[:, :], in_=bias[:, :])

            def evict(ps, m, n):
                ot = opool.tile([P, NF], f32, name="ot", tag="ot")
                nc.vector.tensor_add(
                    ot[:, :], ps[:, :],
                    bias_t[:, n * NF:(n + 1) * NF],
                )
                nc.scalar.dma_start(
                    out=out[m * P:(m + 1) * P, n * NF:(n + 1) * NF],
                    in_=ot[:, :],
                )

            # Stage A: m0..3, two n-pair phases, k-outer chasing the W
            # stream; 8 PSUM banks per phase.
            for phase in range(2):
                pss = {}
                for m in range(MA):
                    for n in range(2):
                        pss[(m, n)] = psum_pool.tile(
                            [P, NF], f32, name=f"psA{phase}_{m}_{n}",
                            tag=f"b{m * 2 + n}")
                for k in range(KT):
                    for m in range(MA):
                        for n in range(2):
                            nc.tensor.matmul(
                                pss[(m, n)][:, :],
                                lhsT=xAt[k][:, m * P:(m + 1) * P],
                                rhs=wc[phase][k][:, n, :],
                                start=(k == 0),
                                stop=(k == KT - 1),
                            )
                for m in range(MA):
                    for n in range(2):
                        evict(pss[(m, n)], m, 2 * phase + n)

            # Stage B: m4..15, k-inner against resident W.
            cnt = 0
            for mi in range(MA, MT):
                for n4 in range(NT):
                    ps = psum_pool.tile([P, NF], f32, name="psB",
                                        tag=f"b{cnt % 8}")
                    cnt += 1
                    for k in range(KT):
                        nc.tensor.matmul(
                            ps[:, :],
                            lhsT=xBt[k][:, (mi - MA) * P:(mi - MA + 1) * P],
                            rhs=wc[n4 // 2][k][:, n4 % 2, :],
                            start=(k == 0),
                            stop=(k == KT - 1),
                        )
                    evict(ps, mi, n4)

    nc.compile()
    return nc


def _get_nc():
    if "nc" not in _CACHE:
        _CACHE["nc"] = _build_nc()
    return _CACHE["nc"]


def _ensure_ntff_hook():
    """Register the axon NTFF profile hook (the image's antenv lacks
    axon_hooks; recreate it and wire the ctypes hook from trn_boot)."""
    import types

    try:
        from antenv.axon_hooks import get_axon_ntff_profile_hook  # noqa: F401
        return
    except ImportError:
        pass
    try:
        import antenv
        from trn_agent_boot.trn_boot import _ntff_profile_via_ctypes

        mod = types.ModuleType("antenv.axon_hooks")
        _state = {"hook": None}
        mod.set_axon_ntff_profile_hook = lambda h: _state.__setitem__("hook", h)
        mod.get_axon_ntff_profile_hook = lambda: _state["hook"]
        sys.modules["antenv.axon_hooks"] = mod
        antenv.axon_hooks = mod
        mod.set_axon_ntff_profile_hook(
            _ntff_profile_via_ctypes("/opt/axon/libaxon_pjrt.so")
        )
        # avoid the S3 artifact upload in the trace path
        import concourse.bass_utils as bu

        bu.upload_artifacts = lambda tmpdir: tmpdir
    except Exception as e:  # profiling is best-effort
        print(f"NTFF hook setup failed: {e}", file=sys.stderr)


def kernel(x, Wg, bg, We, be):
    import ml_dtypes
    from concourse.bass_utils import run_bass_kernel_spmd

    bf16 = ml_dtypes.bfloat16

    x = np.asarray(x, dtype=np.float32)
    Wg = np.asarray(Wg, dtype=np.float32)
    bg = np.asarray(bg, dtype=np.float32)
    We = np.asarray(We, dtype=np.float32)
    be = np.asarray(be, dtype=np.float32)

    # Row-0 gating on host (16K FLOPs): softmax over 8 logits, top-2.
    logits = x[0].astype(np.float64) @ Wg.astype(np.float64).T + bg.astype(
        np.float64
    )
    probs = np.exp(logits - logits.max())
    probs /= probs.sum()
    idx = np.argsort(-probs, kind="stable")[:TOPK]
    w0 = probs[idx]

    Wc = w0[0] * We[idx[0]].astype(np.float64) + w0[1] * We[idx[1]].astype(
        np.float64
    )
    bc = w0[0] * be[idx[0]].astype(np.float64) + w0[1] * be[idx[1]].astype(
        np.float64
    )
    WcT = np.ascontiguousarray(Wc.T)  # [d, o]
    # [ph, k, p, np_, f]: d = k*128 + p, o = ph*1024 + np_*512 + f
    wtb = np.ascontiguousarray(
        WcT.reshape(KT, P, 2, 2, NF).transpose(2, 0, 1, 3, 4)
    ).astype(bf16)
    bias = np.ascontiguousarray(
        np.broadcast_to(bc.astype(np.float32), (P, D))
    )

    nc = _get_nc()
    in_maps = []
    for c in range(N_CORES):
        xsh = x[c * M_SHARD:(c + 1) * M_SHARD]           # [m, d]
        xT = np.ascontiguousarray(xsh.T).astype(bf16)    # [d, m]
        x3 = xT.reshape(KT, P, M_SHARD)                  # [k, p, m]
        xa = np.ascontiguousarray(x3[:, :, :MA * P])
        xb = np.ascontiguousarray(x3[:, :, MA * P:])
        in_maps.append({"xA": xa, "xB": xb, "wt": wtb, "bias": bias})

    trace = bool(int(os.environ.get("KERNEL_TRACE", "0")))
    tmpdir = None
    if trace:
        import tempfile

        _ensure_ntff_hook()
        tmpdir = tempfile.mkdtemp(prefix="moe_trace_")
        _CACHE["last_tmpdir"] = tmpdir
    res = run_bass_kernel_spmd(
        nc, in_maps, core_ids=list(range(N_CORES)), trace=trace, tmpdir=tmpdir
    )
    _CACHE["last_results"] = res

    return np.concatenate(
        [res.results[c]["out"] for c in range(N_CORES)], axis=0
    )
